# revision 22
# baseline (speedup 1.0000x reference)
"""Multi-head attention (B=8, N=1024, C=768, H=12, D=64) on 8 TRN2 NeuronCores.

Strategy: pure data-parallel over batch (B == n_cores == 8), no collectives.
Each core computes full 12-head attention for one batch element, in a fully
transposed layout (channels on SBUF partitions) so no on-device transposes are
needed:

  per core:  xT=[C,N] -> QT,KT=[C,N], V=[N,C] (+ ones col)
             per (head, nk-tile): S^T = K_h Q_h^T  into PSUM [128 nk, 1024 nq]
             S^T += 8*bias^T (DVE);  P^T = exp(0.125*S^T) (ACT -> bf16)
             PV:  [V_h | 1]^T @ P^T -> rows 0:64 = out_h^T (unnorm), row 64 = sum
             softmax sums collected, batch-reciprocal in two halves (overlapped
             with attention), broadcast once per half, normalize on GpSimd,
             out^T = Wp @ attnT + bp -> DMA out, host transposes back.

K/Q projection tiles are interleaved at head boundaries so the TensorEngine
stays dense (HAM un-throttled). Matmuls in bf16 with f32 PSUM accumulation.
"""

import os
import sys
import numpy as np

for _p in ("/opt/trn_rl_repo", "/root/.axon_site/_ro/trn_rl_repo"):
    if os.path.isdir(_p) and _p not in sys.path:
        sys.path.append(_p)

import ml_dtypes

BF16 = ml_dtypes.bfloat16

B, N, C = 8, 1024, 768
H, D = 12, 64
CT = C // 128        # 6 channel tiles
NT = N // 128        # 8 key tiles
F = 512
HA = 10              # heads in normalization batch A (rest in batch B)

_cache = {}


def _build():
    import concourse.bass as bass
    import concourse.tile as tile
    from concourse import bacc, mybir

    f32 = mybir.dt.float32
    bf16 = mybir.dt.bfloat16
    AF = mybir.ActivationFunctionType
    ALU = mybir.AluOpType

    nc = bacc.Bacc("TRN2", target_bir_lowering=False)

    xT_d = nc.dram_tensor("xT", [C, N], bf16, kind="ExternalInput")
    wqT_d = nc.dram_tensor("wqT", [C, C], bf16, kind="ExternalInput")
    wkT_d = nc.dram_tensor("wkT", [C, C], bf16, kind="ExternalInput")
    wvT_d = nc.dram_tensor("wvT", [C, C], bf16, kind="ExternalInput")
    wpT_d = nc.dram_tensor("wpT", [C, C], bf16, kind="ExternalInput")
    bpT_d = nc.dram_tensor("bpT", [128, CT], f32, kind="ExternalInput")
    biasT8_d = nc.dram_tensor("biasT8", [H, N, N], bf16, kind="ExternalInput")
    outT_d = nc.dram_tensor("outT", [C, N], f32, kind="ExternalOutput")
    # softmax-sum scratch: batch A = heads 0..7, batch B = heads 8..11
    sA_scr = nc.dram_tensor("sA_scr", [HA * N], bf16)
    sB_scr = nc.dram_tensor("sB_scr", [(H - HA) * N], bf16)
    rA_scr = nc.dram_tensor("rA_scr", [1, HA * N], bf16)
    rB_scr = nc.dram_tensor("rB_scr", [1, (H - HA) * N], bf16)

    with tile.TileContext(nc) as tc:
        with tc.tile_pool(name="persist", bufs=1) as pers:
            xTb = pers.tile([128, CT, N], bf16, tag="xT")
            wqb = pers.tile([128, CT, C], bf16, tag="wq")
            wkb = pers.tile([128, CT, C], bf16, tag="wk")
            wvb = pers.tile([128, CT, C], bf16, tag="wv")
            wpb = pers.tile([128, CT, C], bf16, tag="wp")
            bpb = pers.tile([128, CT], f32, tag="bp")
            # row 64 collects softmax sums (same start partition as pv row 64)
            s_stage = pers.tile([65, H * N], bf16, tag="s_stage")
            rba = pers.tile([128, H * N], bf16, tag="rba")
            qtb = pers.tile([128, CT, N], bf16, tag="qt")
            ktb = pers.tile([128, CT, N], bf16, tag="kt")
            vb = pers.tile([128, NT, H, D + 1], bf16, tag="v")
            atb = pers.tile([128, CT, N], bf16, tag="at")

            nc.sync.dma_start(
                xTb, xT_d[:].rearrange("(ci p) n -> p ci n", p=128))
            nc.scalar.dma_start(
                wvb, wvT_d[:].rearrange("(ci p) o -> p ci o", p=128))
            nc.sync.dma_start(
                wkb, wkT_d[:].rearrange("(ci p) o -> p ci o", p=128))
            nc.scalar.dma_start(
                wqb, wqT_d[:].rearrange("(ci p) o -> p ci o", p=128))
            nc.sync.dma_start(
                wpb, wpT_d[:].rearrange("(ci p) o -> p ci o", p=128))
            nc.scalar.dma_start(bpb, bpT_d[:])

            nc.vector.memset(vb[:, :, :, D:D + 1], 1.0)

            with tc.tile_pool(name="ups", bufs=2, space="PSUM") as pU, \
                 tc.tile_pool(name="pvps", bufs=2, space="PSUM") as pPV, \
                 tc.tile_pool(name="biasb", bufs=2) as biasp, \
                 tc.tile_pool(name="vstagb", bufs=2) as vstagp, \
                 tc.tile_pool(name="nrmb", bufs=1) as nrm, \
                 tc.tile_pool(name="ptb", bufs=2) as ptp:

                def v_proj(block, nts):
                    f0, fw, h0 = (0, 512, 0) if block == 0 else (512, 256, 8)
                    for nt in nts:
                        ps = pU.tile([128, N], f32, tag="ps")
                        for ci in range(CT):
                            nc.tensor.matmul(
                                ps[:, :fw],
                                lhsT=xTb[:, ci, nt * 128:(nt + 1) * 128],
                                rhs=wvb[:, ci, f0:f0 + fw],
                                start=(ci == 0),
                                stop=(ci == CT - 1),
                            )
                        nc.vector.tensor_copy(
                            vb[:, nt, h0:h0 + fw // D, 0:D],
                            ps[:, :fw].rearrange("p (h d) -> p h d", d=D),
                        )

                def kq_sub(which, cot, nb):
                    wb, dst = (wkb, ktb) if which == "k" else (wqb, qtb)
                    ps = pU.tile([128, N], f32, tag="ps")
                    for ci in range(CT):
                        nc.tensor.matmul(
                            ps[:, :F],
                            lhsT=wb[:, ci, cot * 128:(cot + 1) * 128],
                            rhs=xTb[:, ci, nb * F:(nb + 1) * F],
                            start=(ci == 0),
                            stop=(ci == CT - 1),
                        )
                    nc.vector.tensor_copy(
                        dst[:, cot, nb * F:(nb + 1) * F], ps[:, :F])

                def kq_ct(cot):
                    for which in ("k", "q"):
                        for nb in range(2):
                            kq_sub(which, cot, nb)

                def attn(h):
                    ct, po = h // 2, 64 * (h % 2)
                    bt = biasp.tile([128, NT, N], bf16, tag="bt")
                    nc.sync.dma_start(
                        bt, biasT8_d[h].rearrange("(j p) q -> p j q", p=128))
                    pv0 = pPV.tile([D + 1, F], f32, tag="pv0")
                    pv1 = pPV.tile([D + 1, F], f32, tag="pv1")
                    pvs = (pv0, pv1)
                    for j in range(NT):
                        ps = pU.tile([128, N], f32, tag="ps")
                        for nb in range(2):
                            nc.tensor.matmul(
                                ps[:, nb * F:(nb + 1) * F],
                                lhsT=ktb[po:po + 64, ct, j * 128:(j + 1) * 128],
                                rhs=qtb[po:po + 64, ct, nb * F:(nb + 1) * F],
                                start=True,
                                stop=True,
                            )
                        nc.vector.tensor_tensor(ps, ps, bt[:, j, :], ALU.add)
                        pt = ptp.tile([128, N], bf16, tag="pt")
                        nc.scalar.activation(pt, ps, AF.Exp, scale=0.125)
                        for nb in range(2):
                            nc.tensor.matmul(
                                pvs[nb],
                                lhsT=vb[:, j, h, :],
                                rhs=pt[:, nb * F:(nb + 1) * F],
                                start=(j == 0),
                                stop=(j == NT - 1),
                            )
                    # evacuate pv quickly: unnormalized out^T + softmax sums
                    for nb in range(2):
                        dst = atb[po:po + 64, ct, nb * F:(nb + 1) * F]
                        if po == 0:
                            nc.vector.tensor_copy(dst, pvs[nb][0:D, :])
                        else:
                            vstag = vstagp.tile([D, F], bf16, tag="vstag")
                            nc.vector.tensor_copy(vstag, pvs[nb][0:D, :])
                            nc.gpsimd.dma_start(dst, vstag)
                        nc.scalar.copy(
                            s_stage[D:D + 1, h * N + nb * F:
                                    h * N + (nb + 1) * F],
                            pvs[nb][D:D + 1, :])

                def norm_batch(batch):
                    """Batched reciprocal of softmax sums for a head range."""
                    h0, nh = (0, HA) if batch == 0 else (HA, H - HA)
                    s_scr = sA_scr if batch == 0 else sB_scr
                    r_scr = rA_scr if batch == 0 else rB_scr
                    cols = nh * N // 128
                    nc.scalar.dma_start(
                        s_scr[:], s_stage[D:D + 1, h0 * N:(h0 + nh) * N])
                    sb = nrm.tile([128, H * N // 128], bf16, tag="sb")
                    nc.scalar.dma_start(
                        sb[:, :cols],
                        s_scr[:].rearrange("(p f) -> p f", p=128))
                    rc32 = nrm.tile([128, H * N // 128], f32, tag="rc32")
                    nc.vector.reciprocal(rc32[:, :cols], sb[:, :cols])
                    rcb = nrm.tile([128, H * N // 128], bf16, tag="rcb")
                    nc.vector.tensor_copy(rcb[:, :cols], rc32[:, :cols])
                    nc.scalar.dma_start(
                        r_scr[0, :].rearrange("(p f) -> p f", p=128),
                        rcb[:, :cols])
                    nc.sync.dma_start(
                        rba[:, h0 * N:(h0 + nh) * N],
                        r_scr[:].to_broadcast([128, nh * N]))

                def norm_mul(h):
                    ct, po = h // 2, 64 * (h % 2)
                    sl = atb[po:po + 64, ct, :]
                    nc.gpsimd.tensor_tensor(
                        sl, sl, rba[po:po + 64, h * N:(h + 1) * N], ALU.mult)

                # emission order: kq_ct(ct) fully before attn(2ct); filler
                # spread across head boundaries to keep PE dense
                v_proj(0, range(NT))
                kq_ct(0)
                attn(0)
                kq_sub("k", 1, 0); kq_sub("k", 1, 1)
                attn(1)
                kq_sub("q", 1, 0); kq_sub("q", 1, 1)
                attn(2)
                kq_sub("k", 2, 0); kq_sub("k", 2, 1)
                attn(3)
                kq_sub("q", 2, 0); kq_sub("q", 2, 1)
                v_proj(1, range(0, 4))
                attn(4)
                v_proj(1, range(4, NT))
                kq_sub("k", 3, 0); kq_sub("k", 3, 1)
                attn(5)
                kq_sub("q", 3, 0); kq_sub("q", 3, 1)
                kq_sub("k", 4, 0); kq_sub("k", 4, 1)
                attn(6)
                kq_sub("q", 4, 0); kq_sub("q", 4, 1)
                attn(7)
                kq_sub("k", 5, 0); kq_sub("k", 5, 1)
                attn(8)
                kq_sub("q", 5, 0); kq_sub("q", 5, 1)
                attn(9)
                norm_batch(0)          # heads 0..9: overlapped with attn 10,11
                attn(10)
                norm_mul(0); norm_mul(1); norm_mul(2); norm_mul(3)
                norm_mul(4)
                attn(11)
                norm_mul(5); norm_mul(6); norm_mul(7); norm_mul(8)
                norm_mul(9)
                norm_batch(1)          # heads 10,11
                norm_mul(10); norm_mul(11)

            # ---- output projection ------------------------------------------
            with tc.tile_pool(name="ops", bufs=6, space="PSUM") as pC, \
                 tc.tile_pool(name="otb", bufs=3) as otp:
                for cot in range(CT):
                    for nb in range(2):
                        ps = pC.tile([128, F], f32, tag="o")
                        for ci in range(CT):
                            nc.tensor.matmul(
                                ps,
                                lhsT=wpb[:, ci, cot * 128:(cot + 1) * 128],
                                rhs=atb[:, ci, nb * F:(nb + 1) * F],
                                start=(ci == 0),
                                stop=(ci == CT - 1),
                            )
                        ot = otp.tile([128, F], f32, tag="ot")
                        nc.scalar.activation(
                            ot, ps, AF.Identity, bias=bpb[:, cot:cot + 1])
                        nc.scalar.dma_start(
                            outT_d[cot * 128:(cot + 1) * 128,
                                   nb * F:(nb + 1) * F],
                            ot,
                        )

    nc.compile()
    return nc


def _get_nc():
    if "nc" not in _cache:
        _cache["nc"] = _build()
    return _cache["nc"]


def prep_in_maps(x, attn_bias, Wq, Wk, Wv, Wp, bp):
    """Host-side sharding + layout prep (transposes/casts only)."""
    wqT = np.ascontiguousarray(Wq.T).astype(BF16)
    wkT = np.ascontiguousarray(Wk.T).astype(BF16)
    wvT = np.ascontiguousarray(Wv.T).astype(BF16)
    wpT = np.ascontiguousarray(Wp.T).astype(BF16)
    bpT = np.ascontiguousarray(bp.astype(np.float32).reshape(CT, 128).T)
    biasT8 = np.ascontiguousarray(
        (attn_bias[0].astype(np.float32) * 8.0).transpose(0, 2, 1)
    ).astype(BF16)
    in_maps = []
    for b in range(B):
        in_maps.append({
            "xT": np.ascontiguousarray(x[b].T).astype(BF16),
            "wqT": wqT, "wkT": wkT, "wvT": wvT, "wpT": wpT,
            "bpT": bpT, "biasT8": biasT8,
        })
    return in_maps


def run(in_maps, trace=False, **kw):
    from concourse.bass_utils import run_bass_kernel_spmd

    nc = _get_nc()
    return run_bass_kernel_spmd(
        nc, in_maps, core_ids=list(range(B)), trace=trace, **kw
    )


def kernel(x, attn_bias, Wq, Wk, Wv, Wp, bp):
    res = run(prep_in_maps(x, attn_bias, Wq, Wk, Wv, Wp, bp))
    out = np.stack(
        [res.results[b]["outT"].T for b in range(B)]
    ).astype(np.float32)
    return out


# revision 23
# speedup vs baseline: 1.1620x; 1.1620x over previous
"""Multi-head attention (B=8, N=1024, C=768, H=12, D=64) on 8 TRN2 NeuronCores.

Strategy: pure data-parallel over batch (B == n_cores == 8), no collectives.
Each core computes full 12-head attention for one batch element, in a fully
transposed layout (channels on SBUF partitions) so no on-device transposes are
needed:

  per core:  xT=[C,N] -> QT,KT=[C,N], V=[N,C] (+ ones col)
             per (head, nk-tile): S^T = K_h Q_h^T  into PSUM [128 nk, 1024 nq]
             S^T += 8*bias^T (DVE);  P^T = exp(0.125*S^T) (ACT -> bf16)
             PV:  [V_h | 1]^T @ P^T -> rows 0:64 = out_h^T (unnorm), row 64 = sum
             softmax sums collected, batch-reciprocal in two halves (overlapped
             with attention), broadcast once per half, normalize on GpSimd,
             out^T = Wp @ attnT + bp -> DMA out, host transposes back.

K/Q projection tiles are interleaved at head boundaries so the TensorEngine
stays dense (HAM un-throttled). Matmuls in bf16 with f32 PSUM accumulation.
"""

import os
import sys
import numpy as np

for _p in ("/opt/trn_rl_repo", "/root/.axon_site/_ro/trn_rl_repo"):
    if os.path.isdir(_p) and _p not in sys.path:
        sys.path.append(_p)

import ml_dtypes

BF16 = ml_dtypes.bfloat16

B, N, C = 8, 1024, 768
H, D = 12, 64
CT = C // 128        # 6 channel tiles
NT = N // 128        # 8 key tiles
F = 512
HA = 10              # heads in normalization batch A (rest in batch B)

_cache = {}


def _build():
    import concourse.bass as bass
    import concourse.tile as tile
    from concourse import bacc, mybir

    f32 = mybir.dt.float32
    bf16 = mybir.dt.bfloat16
    AF = mybir.ActivationFunctionType
    ALU = mybir.AluOpType

    nc = bacc.Bacc("TRN2", target_bir_lowering=False)

    xT_d = nc.dram_tensor("xT", [C, N], bf16, kind="ExternalInput")
    wqT_d = nc.dram_tensor("wqT", [C, C], bf16, kind="ExternalInput")
    wkT_d = nc.dram_tensor("wkT", [C, C], bf16, kind="ExternalInput")
    wvT_d = nc.dram_tensor("wvT", [C, C], bf16, kind="ExternalInput")
    wpT_d = nc.dram_tensor("wpT", [C, C], bf16, kind="ExternalInput")
    bpT_d = nc.dram_tensor("bpT", [128, CT], f32, kind="ExternalInput")
    biasT8_d = nc.dram_tensor("biasT8", [H, N, N], bf16, kind="ExternalInput")
    outT_d = nc.dram_tensor("outT", [C, N], f32, kind="ExternalOutput")
    # softmax-sum scratch: batch A = heads 0..7, batch B = heads 8..11
    sA_scr = nc.dram_tensor("sA_scr", [HA * N], bf16)
    sB_scr = nc.dram_tensor("sB_scr", [(H - HA) * N], bf16)
    rA_scr = nc.dram_tensor("rA_scr", [1, HA * N], bf16)
    rB_scr = nc.dram_tensor("rB_scr", [1, (H - HA) * N], bf16)

    with tile.TileContext(nc) as tc:
        with tc.tile_pool(name="persist", bufs=1) as pers:
            xTb = pers.tile([128, CT, N], bf16, tag="xT")
            wqb = pers.tile([128, CT, C], bf16, tag="wq")
            wkb = pers.tile([128, CT, C], bf16, tag="wk")
            wvb = pers.tile([128, CT, C], bf16, tag="wv")
            wpb = pers.tile([128, CT, C], bf16, tag="wp")
            bpb = pers.tile([128, CT], f32, tag="bp")
            # row 64 collects softmax sums (same start partition as pv row 64)
            s_stage = pers.tile([65, H * N], bf16, tag="s_stage")
            rba = pers.tile([128, H * N], bf16, tag="rba")
            qtb = pers.tile([128, CT, N], bf16, tag="qt")
            ktb = pers.tile([128, CT, N], bf16, tag="kt")
            vb = pers.tile([128, NT, H, D + 1], bf16, tag="v")
            atb = pers.tile([128, CT, N], bf16, tag="at")

            nc.sync.dma_start(
                xTb, xT_d[:].rearrange("(ci p) n -> p ci n", p=128))
            nc.scalar.dma_start(
                wvb, wvT_d[:].rearrange("(ci p) o -> p ci o", p=128))
            nc.sync.dma_start(
                wkb, wkT_d[:].rearrange("(ci p) o -> p ci o", p=128))
            nc.scalar.dma_start(
                wqb, wqT_d[:].rearrange("(ci p) o -> p ci o", p=128))
            nc.sync.dma_start(
                wpb, wpT_d[:].rearrange("(ci p) o -> p ci o", p=128))
            nc.scalar.dma_start(bpb, bpT_d[:])

            nc.vector.memset(vb[:, :, :, D:D + 1], 1.0)

            with tc.tile_pool(name="ups", bufs=3, space="PSUM") as pU, \
                 tc.tile_pool(name="pvps", bufs=1, space="PSUM") as pPV, \
                 tc.tile_pool(name="biasb", bufs=2) as biasp, \
                 tc.tile_pool(name="vstagb", bufs=2) as vstagp, \
                 tc.tile_pool(name="nrmb", bufs=1) as nrm, \
                 tc.tile_pool(name="ptb", bufs=2) as ptp:

                def v_proj(block, nts):
                    f0, fw, h0 = (0, 512, 0) if block == 0 else (512, 256, 8)
                    for nt in nts:
                        ps = pU.tile([128, N], f32, tag="ps")
                        for ci in range(CT):
                            nc.tensor.matmul(
                                ps[:, :fw],
                                lhsT=xTb[:, ci, nt * 128:(nt + 1) * 128],
                                rhs=wvb[:, ci, f0:f0 + fw],
                                start=(ci == 0),
                                stop=(ci == CT - 1),
                            )
                        nc.vector.tensor_copy(
                            vb[:, nt, h0:h0 + fw // D, 0:D],
                            ps[:, :fw].rearrange("p (h d) -> p h d", d=D),
                        )

                def kq_sub(which, cot, nb):
                    wb, dst = (wkb, ktb) if which == "k" else (wqb, qtb)
                    ps = pU.tile([128, N], f32, tag="ps")
                    for ci in range(CT):
                        nc.tensor.matmul(
                            ps[:, :F],
                            lhsT=wb[:, ci, cot * 128:(cot + 1) * 128],
                            rhs=xTb[:, ci, nb * F:(nb + 1) * F],
                            start=(ci == 0),
                            stop=(ci == CT - 1),
                        )
                    nc.vector.tensor_copy(
                        dst[:, cot, nb * F:(nb + 1) * F], ps[:, :F])

                def kq_ct(cot):
                    for which in ("k", "q"):
                        for nb in range(2):
                            kq_sub(which, cot, nb)

                def attn(h):
                    ct, po = h // 2, 64 * (h % 2)
                    bt = biasp.tile([128, NT, N], bf16, tag="bt")
                    nc.sync.dma_start(
                        bt, biasT8_d[h].rearrange("(j p) q -> p j q", p=128))
                    pv0 = pPV.tile([D + 1, F], f32, tag="pv0")
                    pv1 = pPV.tile([D + 1, F], f32, tag="pv1")
                    pvs = (pv0, pv1)
                    for j in range(NT):
                        ps = pU.tile([128, N], f32, tag="ps")
                        for nb in range(2):
                            nc.tensor.matmul(
                                ps[:, nb * F:(nb + 1) * F],
                                lhsT=ktb[po:po + 64, ct, j * 128:(j + 1) * 128],
                                rhs=qtb[po:po + 64, ct, nb * F:(nb + 1) * F],
                                start=True,
                                stop=True,
                            )
                        nc.vector.tensor_tensor(ps, ps, bt[:, j, :], ALU.add)
                        pt = ptp.tile([128, N], bf16, tag="pt")
                        nc.scalar.activation(pt, ps, AF.Exp, scale=0.125)
                        for nb in range(2):
                            nc.tensor.matmul(
                                pvs[nb],
                                lhsT=vb[:, j, h, :],
                                rhs=pt[:, nb * F:(nb + 1) * F],
                                start=(j == 0),
                                stop=(j == NT - 1),
                            )
                    # evacuate pv quickly: unnormalized out^T + softmax sums
                    for nb in range(2):
                        dst = atb[po:po + 64, ct, nb * F:(nb + 1) * F]
                        if po == 0:
                            nc.vector.tensor_copy(dst, pvs[nb][0:D, :])
                        else:
                            vstag = vstagp.tile([D, F], bf16, tag="vstag")
                            nc.vector.tensor_copy(vstag, pvs[nb][0:D, :])
                            nc.gpsimd.dma_start(dst, vstag)
                        nc.scalar.copy(
                            s_stage[D:D + 1, h * N + nb * F:
                                    h * N + (nb + 1) * F],
                            pvs[nb][D:D + 1, :])

                def norm_batch(batch):
                    """Batched reciprocal of softmax sums for a head range."""
                    h0, nh = (0, HA) if batch == 0 else (HA, H - HA)
                    s_scr = sA_scr if batch == 0 else sB_scr
                    r_scr = rA_scr if batch == 0 else rB_scr
                    cols = nh * N // 128
                    nc.scalar.dma_start(
                        s_scr[:], s_stage[D:D + 1, h0 * N:(h0 + nh) * N])
                    sb = nrm.tile([128, H * N // 128], bf16, tag="sb")
                    nc.scalar.dma_start(
                        sb[:, :cols],
                        s_scr[:].rearrange("(p f) -> p f", p=128))
                    rc32 = nrm.tile([128, H * N // 128], f32, tag="rc32")
                    nc.vector.reciprocal(rc32[:, :cols], sb[:, :cols])
                    rcb = nrm.tile([128, H * N // 128], bf16, tag="rcb")
                    nc.vector.tensor_copy(rcb[:, :cols], rc32[:, :cols])
                    nc.scalar.dma_start(
                        r_scr[0, :].rearrange("(p f) -> p f", p=128),
                        rcb[:, :cols])
                    nc.sync.dma_start(
                        rba[:, h0 * N:(h0 + nh) * N],
                        r_scr[:].to_broadcast([128, nh * N]))

                def norm_mul(h):
                    ct, po = h // 2, 64 * (h % 2)
                    sl = atb[po:po + 64, ct, :]
                    nc.gpsimd.tensor_tensor(
                        sl, sl, rba[po:po + 64, h * N:(h + 1) * N], ALU.mult)

                # emission order: kq_ct(ct) fully before attn(2ct); filler
                # spread across head boundaries to keep PE dense
                v_proj(0, range(NT))
                kq_ct(0)
                attn(0)
                kq_sub("k", 1, 0); kq_sub("k", 1, 1)
                attn(1)
                kq_sub("q", 1, 0); kq_sub("q", 1, 1)
                attn(2)
                kq_sub("k", 2, 0); kq_sub("k", 2, 1)
                attn(3)
                kq_sub("q", 2, 0); kq_sub("q", 2, 1)
                v_proj(1, range(0, 4))
                attn(4)
                v_proj(1, range(4, NT))
                kq_sub("k", 3, 0); kq_sub("k", 3, 1)
                attn(5)
                kq_sub("q", 3, 0); kq_sub("q", 3, 1)
                kq_sub("k", 4, 0); kq_sub("k", 4, 1)
                attn(6)
                kq_sub("q", 4, 0); kq_sub("q", 4, 1)
                attn(7)
                kq_sub("k", 5, 0); kq_sub("k", 5, 1)
                attn(8)
                kq_sub("q", 5, 0); kq_sub("q", 5, 1)
                attn(9)
                norm_batch(0)          # heads 0..9: overlapped with attn 10,11
                attn(10)
                norm_mul(0); norm_mul(1); norm_mul(2); norm_mul(3)
                norm_mul(4)
                attn(11)
                norm_mul(5); norm_mul(6); norm_mul(7); norm_mul(8)
                norm_mul(9)
                norm_batch(1)          # heads 10,11
                norm_mul(10); norm_mul(11)

            # ---- output projection ------------------------------------------
            with tc.tile_pool(name="ops", bufs=6, space="PSUM") as pC, \
                 tc.tile_pool(name="otb", bufs=3) as otp:
                for cot in range(CT):
                    for nb in range(2):
                        ps = pC.tile([128, F], f32, tag="o")
                        for ci in range(CT):
                            nc.tensor.matmul(
                                ps,
                                lhsT=wpb[:, ci, cot * 128:(cot + 1) * 128],
                                rhs=atb[:, ci, nb * F:(nb + 1) * F],
                                start=(ci == 0),
                                stop=(ci == CT - 1),
                            )
                        ot = otp.tile([128, F], f32, tag="ot")
                        nc.scalar.activation(
                            ot, ps, AF.Identity, bias=bpb[:, cot:cot + 1])
                        nc.scalar.dma_start(
                            outT_d[cot * 128:(cot + 1) * 128,
                                   nb * F:(nb + 1) * F],
                            ot,
                        )

    nc.compile()
    return nc


def _get_nc():
    if "nc" not in _cache:
        _cache["nc"] = _build()
    return _cache["nc"]


def prep_in_maps(x, attn_bias, Wq, Wk, Wv, Wp, bp):
    """Host-side sharding + layout prep (transposes/casts only)."""
    wqT = np.ascontiguousarray(Wq.T).astype(BF16)
    wkT = np.ascontiguousarray(Wk.T).astype(BF16)
    wvT = np.ascontiguousarray(Wv.T).astype(BF16)
    wpT = np.ascontiguousarray(Wp.T).astype(BF16)
    bpT = np.ascontiguousarray(bp.astype(np.float32).reshape(CT, 128).T)
    biasT8 = np.ascontiguousarray(
        (attn_bias[0].astype(np.float32) * 8.0).transpose(0, 2, 1)
    ).astype(BF16)
    in_maps = []
    for b in range(B):
        in_maps.append({
            "xT": np.ascontiguousarray(x[b].T).astype(BF16),
            "wqT": wqT, "wkT": wkT, "wvT": wvT, "wpT": wpT,
            "bpT": bpT, "biasT8": biasT8,
        })
    return in_maps


def run(in_maps, trace=False, **kw):
    from concourse.bass_utils import run_bass_kernel_spmd

    nc = _get_nc()
    return run_bass_kernel_spmd(
        nc, in_maps, core_ids=list(range(B)), trace=trace, **kw
    )


def kernel(x, attn_bias, Wq, Wk, Wv, Wp, bp):
    res = run(prep_in_maps(x, attn_bias, Wq, Wk, Wv, Wp, bp))
    out = np.stack(
        [res.results[b]["outT"].T for b in range(B)]
    ).astype(np.float32)
    return out
